# revision 10
# baseline (speedup 1.0000x reference)
import os
import numpy as np
import ml_dtypes
BISECT = int(os.environ.get('BISECT', '9'))
LAST_EXEC_NS = None

H = 128
OUT = 128
NB = 8
SBF_D = 42
NR = 6
E = 50000
T = 200000
NCORES = 8
ES = E // NCORES          # 6250 edges per core
EP = 6656                 # padded edge count per core (13 * 512)
AGG_ROWS = EP + 16        # scatter table rows; dump row below
DUMP_ROW = EP + 1
WE = 32                   # edge window per chunk
GRP = 4                   # chunks per group


def _silu(x):
    return x / (1.0 + np.exp(-x))


def _prep_core(order_idx, idx_ji_l, idx_kj_g, sbf_s):
    """Chunk one core's triplets (sorted by local edge id).
    Returns per-chunk arrays. idx_ji_l: local edge ids sorted ascending."""
    nt = len(idx_ji_l)
    # segment starts per edge
    starts = np.searchsorted(idx_ji_l, np.arange(ES + 1))
    chunks = []   # (tri_lo, tri_hi, base_e, n_e)
    e = 0
    while e < ES:
        base = e
        t_lo = starts[e]
        n_e = 0
        while e < ES and n_e < WE:
            seg = starts[e + 1] - starts[e]
            if seg > 128:
                raise RuntimeError("segment > 128 triplets unsupported")
            if starts[e + 1] - t_lo > 128:
                break
            e += 1
            n_e += 1
        chunks.append((t_lo, starts[e], base, e - base))
    return chunks


def _build_host_data(x, rbf, sbf, idx_kj, idx_ji):
    bf16 = ml_dtypes.bfloat16
    order = np.argsort(idx_ji, kind="stable")
    ji_s = idx_ji[order]
    kj_s = idx_kj[order]
    core_lo = np.searchsorted(ji_s, np.arange(0, E + 1, ES))

    per_core = []
    for c in range(NCORES):
        lo, hi = core_lo[c], core_lo[c + 1]
        ji_l = (ji_s[lo:hi] - c * ES).astype(np.int64)
        kj_c = kj_s[lo:hi]
        ord_c = order[lo:hi]
        # insert dummy triplets for empty edges
        cnt = np.bincount(ji_l, minlength=ES)
        missing = np.where(cnt == 0)[0]
        if len(missing):
            ji_l = np.concatenate([ji_l, missing])
            kj_c = np.concatenate([kj_c, np.zeros(len(missing), np.int64)])
            ord_c = np.concatenate([ord_c, np.full(len(missing), -1)])
            o2 = np.argsort(ji_l, kind="stable")
            ji_l, kj_c, ord_c = ji_l[o2], kj_c[o2], ord_c[o2]
        chunks = _prep_core(ord_c, ji_l, kj_c, None)
        per_core.append((chunks, ji_l, kj_c, ord_c))

    nch = max(len(pc[0]) for pc in per_core)
    nch = ((nch + GRP - 1) // GRP) * GRP
    ngrp = nch // GRP

    sbfT_all = np.zeros((NCORES, nch, SBF_D, 128), bf16)
    oh_all = np.zeros((NCORES, nch, 128, WE), bf16)
    idx_all = np.zeros((NCORES, nch, 128, 1), np.int32)
    scat_all = np.full((NCORES, ngrp, 128, 1), DUMP_ROW, np.int32)

    sbf_b = sbf.astype(bf16)
    for c in range(NCORES):
        chunks, ji_l, kj_c, ord_c = per_core[c]
        for ci, (t_lo, t_hi, base, n_e) in enumerate(chunks):
            n = t_hi - t_lo
            tri = ord_c[t_lo:t_hi]            # global triplet ids (-1 = dummy)
            real = tri >= 0
            rows = np.zeros((n, SBF_D), bf16)
            rows[real] = sbf_b[tri[real]]
            sbfT_all[c, ci, :, :n] = rows.T
            idx_all[c, ci, :n, 0] = kj_c[t_lo:t_hi]
            el = ji_l[t_lo:t_hi] - base
            oh_all[c, ci, np.arange(n), el] = 1
            g, cc = divmod(ci, GRP)
            sl = slice(cc * WE, cc * WE + n_e)
            scat_all[c, g, sl, 0] = np.arange(base, base + n_e) + 0
    return nch, ngrp, sbfT_all, oh_all, idx_all, scat_all


def kernel(x, rbf, sbf, idx_kj, idx_ji, W_rbf, W_sbf, Wkj, bkj, Wji, bji, Wbil,
           before_W1, before_b1, before_W2, before_b2, Wlin, blin,
           after_W1, after_b1, after_W2, after_b2, Wout, bout):
    import concourse.bass as bass
    import concourse.bacc as bacc
    import concourse.mybir as mybir
    import concourse.tile as tile
    from concourse import bass_utils

    bf16 = ml_dtypes.bfloat16
    f32 = np.float32
    x = np.asarray(x, f32); rbf = np.asarray(rbf, f32); sbf = np.asarray(sbf, f32)
    idx_kj = np.asarray(idx_kj).astype(np.int64)
    idx_ji = np.asarray(idx_ji).astype(np.int64)

    nch, ngrp, sbfT_all, oh_all, idx_all, scat_all = _build_host_data(
        x, rbf, sbf, idx_kj, idx_ji)

    # per-core inputs
    xT32s, xTbs, rbfTbs = [], [], []
    for c in range(NCORES):
        xs = np.zeros((128, EP), f32)
        xs[:, :ES] = x[c * ES:(c + 1) * ES].T
        xT32s.append(xs)
        xTbs.append(xs.astype(bf16))
        rs = np.zeros((NR, EP), bf16)
        rs[:, :ES] = rbf[c * ES:(c + 1) * ES].T.astype(bf16)
        rbfTbs.append(rs)

    wb_all = np.ascontiguousarray(
        np.transpose(Wbil, (2, 1, 0))).astype(bf16)       # [l, j, i]
    wts = {
        "w_kj": np.asarray(Wkj, f32).astype(bf16), "w_ji": np.asarray(Wji, f32).astype(bf16),
        "w_rbf": np.asarray(W_rbf, f32).astype(bf16), "w_sbf": np.asarray(W_sbf, f32).astype(bf16),
        "w_b1": np.asarray(before_W1[0], f32).astype(bf16), "w_b2": np.asarray(before_W2[0], f32).astype(bf16),
        "w_lin": np.asarray(Wlin, f32).astype(bf16),
        "w_a1_0": np.asarray(after_W1[0], f32).astype(bf16), "w_a2_0": np.asarray(after_W2[0], f32).astype(bf16),
        "w_a1_1": np.asarray(after_W1[1], f32).astype(bf16), "w_a2_1": np.asarray(after_W2[1], f32).astype(bf16),
        "w_out": np.asarray(Wout, f32).astype(bf16),
    }
    biases = {
        "b_kj": np.asarray(bkj, f32), "b_ji": np.asarray(bji, f32),
        "b_b1": np.asarray(before_b1[0], f32), "b_b2": np.asarray(before_b2[0], f32),
        "b_lin": np.asarray(blin, f32),
        "b_a1_0": np.asarray(after_b1[0], f32), "b_a2_0": np.asarray(after_b2[0], f32),
        "b_a1_1": np.asarray(after_b1[1], f32), "b_a2_1": np.asarray(after_b2[1], f32),
        "b_out": np.asarray(bout, f32),
    }

    nc = bacc.Bacc(None, target_bir_lowering=False, num_devices=NCORES)
    dt = mybir.dt
    ACT = mybir.ActivationFunctionType

    t_xT32 = nc.dram_tensor("xT32", [128, EP], dt.float32, kind="ExternalInput")
    t_xTb = nc.dram_tensor("xTb", [128, EP], dt.bfloat16, kind="ExternalInput")
    t_rbfTb = nc.dram_tensor("rbfTb", [NR, EP], dt.bfloat16, kind="ExternalInput")
    t_sbfT = nc.dram_tensor("sbfT", [nch, SBF_D, 128], dt.bfloat16, kind="ExternalInput")
    t_oh = nc.dram_tensor("oh", [nch, 128, WE], dt.bfloat16, kind="ExternalInput")
    t_idx = nc.dram_tensor("idx", [nch, 128, 1], dt.int32, kind="ExternalInput")
    t_scat = nc.dram_tensor("scat", [ngrp, 128, 1], dt.int32, kind="ExternalInput")
    t_w = {k: nc.dram_tensor(k, list(v.shape), dt.bfloat16, kind="ExternalInput")
           for k, v in wts.items()}
    t_b = {k: nc.dram_tensor(k, [128, 1], dt.float32, kind="ExternalInput")
           for k in biases}
    t_wb = nc.dram_tensor("wb", [128, NB, 128], dt.bfloat16, kind="ExternalInput")
    t_out = nc.dram_tensor("outT", [128, EP], dt.float32, kind="ExternalOutput")

    NT1 = 49  # phase-1 row tiles (49*128 = 6272 >= 6250)

    with tile.TileContext(nc) as tc:
        with (
            tc.tile_pool(name="const", bufs=1) as cpool,
            tc.tile_pool(name="dram", bufs=1, space="DRAM") as dpool,
            tc.tile_pool(name="big", bufs=1) as bigpool,
        ):
            # load weights/biases to SBUF
            w_sb = {}
            for k, tt in t_w.items():
                w_sb[k] = cpool.tile(list(tt.shape), dt.bfloat16, tag=k, name=f"w_{k}")
                nc.sync.dma_start(w_sb[k][:], tt[:])
            wb_sb = cpool.tile([128, NB, 128], dt.bfloat16, tag="wb")
            nc.sync.dma_start(wb_sb[:], t_wb[:])
            b_sb = {}
            for k in t_b:
                b_sb[k] = cpool.tile([128, 1], dt.float32, tag=k, name=f"bs_{k}")
                nc.sync.dma_start(b_sb[k][:], t_b[k][:])
            xTb_sb = bigpool.tile([128, EP], dt.bfloat16, tag="xTb")
            nc.sync.dma_start(xTb_sb[:], t_xTb[:])
            rbfT_sb = cpool.tile([NR, EP], dt.bfloat16, tag="rbfT")
            nc.sync.dma_start(rbfT_sb[:], t_rbfTb[:])

            kj_shard = dpool.tile([ES, 128], dt.bfloat16, tag="kjshard")
            kj_full = dpool.tile([E, 128], dt.bfloat16, tag="kjfull")
            agg_d = dpool.tile([AGG_ROWS, 128], dt.bfloat16, tag="aggd")

            use_bkj = bool(np.any(biases["b_kj"]))
            bkj_row = None
            if use_bkj:
                bkj_row = cpool.tile([1, 128], dt.float32, tag="bkjrow")
                # bias along free dim for row-layout tiles
                nc.sync.dma_start(bkj_row[:], t_b["b_kj"].rearrange("p one -> one p"))

            # ---- phase 1: x_kj shard in row layout ----
            kj_rows = bigpool.tile([128, NT1, 128], dt.bfloat16, tag="kjrows")
            with (
                tc.tile_pool(name="p1ps", bufs=4, space="PSUM") as p1ps,
                tc.tile_pool(name="p1sb", bufs=4) as p1sb,
            ):
                for t in range(NT1):
                    ps_x = p1ps.tile([128, 128], dt.float32, tag="psx")
                    nc.tensor.matmul(ps_x[:], xTb_sb[:, t * 128:(t + 1) * 128],
                                     w_sb["w_kj"][:], start=True, stop=True)
                    ps_r = p1ps.tile([128, 128], dt.float32, tag="psr")
                    nc.tensor.matmul(ps_r[:], rbfT_sb[:, t * 128:(t + 1) * 128],
                                     w_sb["w_rbf"][:], start=True, stop=True)
                    sl_t = p1sb.tile([128, 128], dt.bfloat16, tag="silu")
                    if use_bkj:
                        nc.vector.tensor_tensor(
                            out=ps_x[:], in0=ps_x[:],
                            in1=bkj_row[:].to_broadcast([128, 128]),
                            op=mybir.AluOpType.add)
                    nc.scalar.activation(sl_t[:], ps_x[:], ACT.Silu)
                    nc.vector.tensor_tensor(out=kj_rows[:, t, :], in0=sl_t[:],
                                            in1=ps_r[:], op=mybir.AluOpType.mult)
            # DMA shard out: kj_shard rows e = 128*t + p
            for t in range(NT1):
                r0 = t * 128
                r1 = min(r0 + 128, ES)
                if r0 >= ES:
                    break
                nc.sync.dma_start(kj_shard[r0:r1, :], kj_rows[:r1 - r0, t, :])

            if BISECT >= 2:
                nc.gpsimd.collective_compute(
                    "AllGather", mybir.AluOpType.bypass,
                    replica_groups=[list(range(NCORES))],
                    ins=[kj_shard.opt()], outs=[kj_full.opt()],
                )
            else:
                nc.sync.dma_start(kj_full[:ES, :], kj_shard[:])

            # ---- x_jiT ----
            xji_sb = bigpool.tile([128, EP], dt.bfloat16, tag="xji")
            with tc.tile_pool(name="p1bps", bufs=4, space="PSUM") as pps:
                for s in range(EP // 512):
                    ps = pps.tile([128, 512], dt.float32, tag="ps")
                    nc.tensor.matmul(ps[:], w_sb["w_ji"][:],
                                     xTb_sb[:, s * 512:(s + 1) * 512],
                                     start=True, stop=True)
                    nc.scalar.activation(xji_sb[:, s * 512:(s + 1) * 512], ps[:],
                                         ACT.Silu, bias=b_sb["b_ji"][:])

            # ---- phase 2 ----
            with (
                tc.tile_pool(name="p2in", bufs=6) as p2in,
                tc.tile_pool(name="p2ps", bufs=2, space="PSUM") as p2ps,
                tc.tile_pool(name="p2sb", bufs=3) as p2sb,
            ):
                for g in range(ngrp):
                    sbfh_ps = p2ps.tile([128, GRP * NB], dt.float32, tag="sbfh")
                    gt_sb = p2sb.tile([128, NB, GRP, WE], dt.bfloat16, tag="gt")
                    for cc in range(GRP):
                        ch = g * GRP + cc
                        sbfT_t = p2in.tile([SBF_D, 128], dt.bfloat16, tag="sbft")
                        nc.sync.dma_start(sbfT_t[:], t_sbfT[ch])
                        oh_t = p2in.tile([128, WE], dt.bfloat16, tag="oht")
                        nc.sync.dma_start(oh_t[:], t_oh[ch])
                        idx_t = p2in.tile([128, 1], dt.int32, tag="idxt")
                        nc.sync.dma_start(idx_t[:], t_idx[ch])
                        xg_t = p2in.tile([128, 128], dt.bfloat16, tag="xgt")
                        if BISECT >= 3:
                            nc.gpsimd.indirect_dma_start(
                                out=xg_t[:], out_offset=None,
                                in_=kj_full[:],
                                in_offset=bass.IndirectOffsetOnAxis(ap=idx_t[:, :1], axis=0),
                            )
                        else:
                            nc.sync.dma_start(xg_t[:], kj_full[:128, :])
                        nc.tensor.matmul(sbfh_ps[:, cc * NB:(cc + 1) * NB],
                                         sbfT_t[:], w_sb["w_sbf"][:],
                                         start=True, stop=True)
                        ohs_t = p2sb.tile([128, NB, WE], dt.bfloat16, tag="ohs")
                        for j in range(NB):
                            nc.vector.tensor_scalar(
                                out=ohs_t[:, j, :], in0=oh_t[:],
                                scalar1=sbfh_ps[:, cc * NB + j:cc * NB + j + 1],
                                scalar2=None, op0=mybir.AluOpType.mult)
                        g_ps = p2ps.tile([128, NB * WE], dt.float32, tag="gps")
                        nc.tensor.matmul(g_ps[:], xg_t[:],
                                         ohs_t[:].rearrange("p j e -> p (j e)"),
                                         start=True, stop=True)
                        nc.scalar.activation(
                            gt_sb[:, :, cc, :],
                            g_ps[:].rearrange("p (j e) -> p j e", j=NB), ACT.Copy)
                    agg_ps = p2ps.tile([128, 128], dt.float32, tag="aggps")
                    for j in range(NB):
                        nc.tensor.matmul(
                            agg_ps[:],
                            gt_sb[:, j],
                            wb_sb[:, j, :], start=(j == 0), stop=(j == NB - 1))
                    agg_sb = p2sb.tile([128, 128], dt.bfloat16, tag="aggsb")
                    nc.vector.tensor_copy(agg_sb[:], agg_ps[:])
                    scat_t = p2in.tile([128, 1], dt.int32, tag="scat")
                    nc.sync.dma_start(scat_t[:], t_scat[g])
                    if BISECT >= 4:
                        nc.gpsimd.indirect_dma_start(
                            out=agg_d[:], out_offset=bass.IndirectOffsetOnAxis(
                                ap=scat_t[:, :1], axis=0),
                            in_=agg_sb[:], in_offset=None,
                        )
                    else:
                        nc.sync.dma_start(agg_d[g * 128:(g + 1) * 128, :] if (g + 1) * 128 <= AGG_ROWS else agg_d[:128, :], agg_sb[:])

            # ---- phase 3 ----
            aggT = bigpool.tile([128, EP], dt.bfloat16, tag="aggT")
            if BISECT >= 5:
                nc.sync.dma_start_transpose(aggT[:], agg_d[:EP, :])
            else:
                nc.gpsimd.memset(aggT[:], 0)
            hT = bigpool.tile([128, EP], dt.bfloat16, tag="hT")
            nc.vector.tensor_tensor(out=hT[:], in0=xji_sb[:], in1=aggT[:],
                                    op=mybir.AluOpType.add)

            def layer(dst, w_key, b_key, src):
                with tc.tile_pool(name=f"ps_{w_key}", bufs=4, space="PSUM") as pps:
                    for s in range(EP // 512):
                        ps = pps.tile([128, 512], dt.float32, tag="ps")
                        nc.tensor.matmul(ps[:], w_sb[w_key][:],
                                         src[:, s * 512:(s + 1) * 512],
                                         start=True, stop=True)
                        nc.scalar.activation(dst[:, s * 512:(s + 1) * 512], ps[:],
                                             ACT.Silu, bias=b_sb[b_key][:])

            tmp1 = bigpool.tile([128, EP], dt.bfloat16, tag="tmp1")
            tmp2 = bigpool.tile([128, EP], dt.bfloat16, tag="tmp2")

            # before block
            layer(tmp1, "w_b1", "b_b1", hT)
            layer(tmp2, "w_b2", "b_b2", tmp1)
            nc.vector.tensor_tensor(out=hT[:], in0=hT[:], in1=tmp2[:],
                                    op=mybir.AluOpType.add)
            # lin + residual x
            layer(tmp1, "w_lin", "b_lin", hT)
            nc.vector.tensor_tensor(out=hT[:], in0=tmp1[:], in1=xTb_sb[:],
                                    op=mybir.AluOpType.add)
            # after blocks
            for a in range(2):
                layer(tmp1, f"w_a1_{a}", f"b_a1_{a}", hT)
                layer(tmp2, f"w_a2_{a}", f"b_a2_{a}", tmp1)
                nc.vector.tensor_tensor(out=hT[:], in0=hT[:], in1=tmp2[:],
                                        op=mybir.AluOpType.add)
            # out layer -> f32
            out_sb = bigpool.tile([128, EP], dt.float32, tag="outsb")
            with tc.tile_pool(name="ps_out", bufs=4, space="PSUM") as pps:
                for s in range(EP // 512):
                    ps = pps.tile([128, 512], dt.float32, tag="ps")
                    nc.tensor.matmul(ps[:], w_sb["w_out"][:],
                                     hT[:, s * 512:(s + 1) * 512],
                                     start=True, stop=True)
                    nc.scalar.activation(out_sb[:, s * 512:(s + 1) * 512], ps[:],
                                         ACT.Silu, bias=b_sb["b_out"][:])
            nc.sync.dma_start(t_out[:], out_sb[:])

    in_maps = []
    for c in range(NCORES):
        m = {"xT32": xT32s[c], "xTb": xTbs[c], "rbfTb": rbfTbs[c],
             "sbfT": np.ascontiguousarray(sbfT_all[c]),
             "oh": np.ascontiguousarray(oh_all[c]),
             "idx": np.ascontiguousarray(idx_all[c]),
             "scat": np.ascontiguousarray(scat_all[c]),
             "wb": wb_all}
        m.update(wts)
        for k, v in biases.items():
            m[k] = np.ascontiguousarray(v.reshape(128, 1))
        in_maps.append(m)

    nc.compile()
    import time as _time
    t0 = _time.time()
    res = bass_utils.run_bass_kernel_spmd(
        nc, in_maps, core_ids=list(range(NCORES)))
    global LAST_EXEC_NS
    LAST_EXEC_NS = res.exec_time_ns
    if LAST_EXEC_NS is None:
        LAST_EXEC_NS = int((_time.time() - t0) * 1e9)
    outs = [r["outT"][:, :ES].T for r in res.results]
    return np.concatenate(outs, axis=0).astype(np.float32)


if __name__ == "__main__":
    import reference
    inp = {k: np.asarray(v) for k, v in reference.setup_inputs().items()}
    out = kernel(**inp)
    exp = np.asarray(reference.reference(**inp))
    err = np.abs(out - exp).max() / (np.abs(exp).max() + 1e-9)
    print("rel err:", err)
